# revision 21
# baseline (speedup 1.0000x reference)
"""Trainium2 Bass kernel for nn_GAT_49168785605294.

GRU(4096) recurrence, T=64 steps, batch=1, followed by a 4-layer MLP head.

Restructuring (exact math):
  - x = (temp @ W1 + b1).flatten() is rank-1 in temp  ->  fold W1/b1 into Wih:
        gi = temp @ A + cI,   A[n,j] = sum_l W1[0,l] Wih[32n+l, j]   (A: [128, 12288])
    This removes the 201MB Wih entirely (32x less matvec work for the i-gates).
  - Gate-dim (3*4096) column-sharded across 8 cores (512 cols per gate per core).
    Whh shard (4096 x 1536, fp16) is SBUF-resident on each core.
  - Per step: PE matvec in "transposed" orientation (out tiles [128,1], weights
    as stationary lhsT) so gates land partition-major for DVE/ACT; block-diag
    W2 / Wh2i transforms as tiny matmuls; AllGather of (hidden'[512], temp'[16])
    per step across the 8 cores.
  - Head (Wh1/Wh2a/Wh3/Whh2) column-sharded, fp32, with 3 small AllGathers;
    the final [2048] output is assembled host-side from per-core shards.
"""
import numpy as np

import concourse.bass as bass
import concourse.mybir as mybir
import concourse.tile as tile
from concourse import bacc, bass_utils

F32 = mybir.dt.float32
F16 = mybir.dt.float16

NC = 8            # cores
NODES = 128
LATENT = 32
GRU = 4096        # = NODES * LATENT
HID = 1024
T_IN = 32
T = 64
CPG = 512         # columns per gate per core
MT = 12           # m-tiles per core per step (1536 / 128)
KT = 32           # k-tiles of the 4096 hidden dim
WCOLS = KT * 3 * CPG  # 49152 fp16 free elems of whh per partition

AF = mybir.ActivationFunctionType


def build(t_steps=T, debug_taps=False, skip_whh=False, skip_ag=False):
    nc = bacc.Bacc("TRN2", target_bir_lowering=False, debug=False,
                   enable_asserts=False, num_devices=NC)

    # ---- kernel I/O (per core) ----
    din = {}
    def inp(name, shape, dt):
        din[name] = nc.dram_tensor(name, list(shape), dt, kind="ExternalInput").ap()
    inp("whh", [128, WCOLS], F16)       # Whh shard, k-tile-major lhsT layout
    inp("a16", [128, 3 * CPG], F16)     # A shard
    inp("gb", [128, 16], F32)           # gate biases in psum_g layout
    inp("b2t", [128, 1], F32)           # tiled b2
    inp("bd", [128, 128], F16)          # I4 (x) W2
    inp("bd2", [128, 4], F16)           # I4 (x) Wh2i
    inp("h16", [128, T_IN], F16)        # teacher-forcing inputs, h16[n,t]
    inp("wh1", [128, 32 * 128], F32)    # head stage 1 row-shard (lhsT layout)
    inp("bh1f", [128, 8], F32)          # bh1 in p-major layout
    inp("wh2a", [128, 8 * 128], F32)
    inp("bs2", [128, 1], F32)           # includes folded table @ Wh2a_bot
    inp("wh3", [128, 8 * 128], F32)
    inp("bs3", [128, 1], F32)
    inp("whh2", [128, 8 * 256], F32)
    inp("bs4", [128, 2], F32)
    inp("consts", [128, 1], F32)        # bh2i broadcast
    zout = nc.dram_tensor("zout", [256], F32, kind="ExternalOutput").ap()
    if debug_taps:
        dbg_hp = nc.dram_tensor("dbg_hp", [128, 4], F32, kind="ExternalOutput").ap()
        dbg_g = nc.dram_tensor("dbg_g", [128, 16], F32, kind="ExternalOutput").ap()
        dbg_hv1 = nc.dram_tensor("dbg_hv1", [128, 8], F32, kind="ExternalOutput").ap()
        dbg_v1 = nc.dram_tensor("dbg_v1", [128, 8], F32, kind="ExternalOutput").ap()

    # ---- collective buffers ----
    ag_in = [nc.dram_tensor(f"ag_in{i}", [528], F32).ap() for i in range(2)]
    ag_out = [nc.dram_tensor(f"ag_out{i}", [8, 528], F32, addr_space="Shared").ap()
              for i in range(2)]
    tmp_dram = [nc.dram_tensor(f"tmp_dram{i}", [128], F32).ap() for i in range(2)]
    ar_in = nc.dram_tensor("ar_in", [1024], F32).ap()
    ar_out = nc.dram_tensor("ar_out", [1024], F32, addr_space="Shared").ap()
    agh_in = [nc.dram_tensor(f"agh_in{i}", [128], F32).ap() for i in range(3)]
    agh_out = [nc.dram_tensor(f"agh_out{i}", [8, 128], F32, addr_space="Shared").ap()
               for i in range(3)]
    RG = [list(range(NC))]

    with tile.TileContext(nc) as tc:
        with (
            tc.tile_pool(name="wpool", bufs=1) as wpool,
            tc.tile_pool(name="cpool", bufs=1) as cpool,
            tc.tile_pool(name="spool", bufs=3) as spool,
            tc.tile_pool(name="hpool", bufs=2) as hpool,
            tc.tile_pool(name="pg", bufs=2, space="PSUM") as pgp,
            tc.tile_pool(name="pbd", bufs=2, space="PSUM") as pbdp,
            tc.tile_pool(name="ptp", bufs=2, space="PSUM") as ptpp,
        ):
            # ---- load weights/consts to SBUF ----
            whh_sb = wpool.tile([128, WCOLS], F16, tag="whh")
            NSPLIT = 32
            csz = WCOLS // NSPLIT
            for i in range(NSPLIT):
                nc.sync.dma_start(whh_sb[:, i * csz:(i + 1) * csz],
                                  din["whh"][:, i * csz:(i + 1) * csz])
            a_sb = cpool.tile([128, 3 * CPG], F16, tag="a16")
            nc.sync.dma_start(a_sb[:], din["a16"][:])
            gb_sb = cpool.tile([128, 16], F32, tag="gb")
            nc.sync.dma_start(gb_sb[:], din["gb"][:])
            b2t_sb = cpool.tile([128, 1], F32, tag="b2t")
            nc.sync.dma_start(b2t_sb[:], din["b2t"][:])
            bd_sb = cpool.tile([128, 128], F16, tag="bd")
            nc.sync.dma_start(bd_sb[:], din["bd"][:])
            bd2_sb = cpool.tile([128, 4], F16, tag="bd2")
            nc.sync.dma_start(bd2_sb[:], din["bd2"][:])
            h16_sb = cpool.tile([128, T_IN], F16, tag="h16")
            nc.sync.dma_start(h16_sb[:], din["h16"][:])
            cons_sb = cpool.tile([128, 1], F32, tag="consts")
            nc.sync.dma_start(cons_sb[:], din["consts"][:])

            wh1_sb = wpool.tile([128, 32 * 128], F32, tag="wh1")
            for i in range(8):
                nc.sync.dma_start(wh1_sb[:, i * 512:(i + 1) * 512],
                                  din["wh1"][:, i * 512:(i + 1) * 512])
            wh2a_sb = wpool.tile([128, 8 * 128], F32, tag="wh2a")
            nc.sync.dma_start(wh2a_sb[:], din["wh2a"][:])
            wh3_sb = wpool.tile([128, 8 * 128], F32, tag="wh3")
            nc.sync.dma_start(wh3_sb[:], din["wh3"][:])
            whh2_sb = wpool.tile([128, 8 * 256], F32, tag="whh2")
            for i in range(4):
                nc.sync.dma_start(whh2_sb[:, i * 512:(i + 1) * 512],
                                  din["whh2"][:, i * 512:(i + 1) * 512])
            bh1f_sb = cpool.tile([128, 8], F32, tag="bh1f")
            nc.sync.dma_start(bh1f_sb[:], din["bh1f"][:])
            bs2_sb = cpool.tile([128, 1], F32, tag="bs2")
            nc.sync.dma_start(bs2_sb[:], din["bs2"][:])
            bs3_sb = cpool.tile([128, 1], F32, tag="bs3")
            nc.sync.dma_start(bs3_sb[:], din["bs3"][:])
            bs4_sb = cpool.tile([128, 2], F32, tag="bs4")
            nc.sync.dma_start(bs4_sb[:], din["bs4"][:])

            # ---- recurrence ----
            hp_prev = None        # [128,4] f32, own shard of current hidden
            hid16 = None          # [128,32] f16, full hidden (rhs for Whh MMs)
            tmp16 = None          # [128,1]  f16, full temp

            for t in range(t_steps):
                pg = pgp.tile([128, 16], F32, tag="pg")
                # --- matvec into psum_g ---
                # cols 0-3: G_r = gi_r + gh_r ; cols 4-7: G_z ; 8-11: gh_n ; 12-15: gi_n
                rhs_a = h16_sb[:, t:t + 1] if t < T_IN else tmp16[:, 0:1]
                for m in range(MT):
                    pcol = m if m < 8 else m + 4
                    nc.tensor.matmul(pg[:, pcol:pcol + 1],
                                     a_sb[:, m * 128:(m + 1) * 128], rhs_a,
                                     start=(m == 0),
                                     stop=((t == 0 or skip_whh) and m == MT - 1))
                if t > 0 and not skip_whh:
                    loop = ([(m, k) for k in range(KT) for m in range(MT)]
                            if t == 1 else
                            [(m, k) for m in range(MT) for k in range(KT)])
                    for m, k in loop:
                        nc.tensor.matmul(
                            pg[:, m:m + 1],
                            whh_sb[:, (k * 3 * CPG + m * 128):
                                      (k * 3 * CPG + m * 128 + 128)],
                            hid16[:, k:k + 1],
                            start=False,
                            stop=(m, k) == loop[-1])

                # --- gates (DVE/ACT) ---
                gsb = spool.tile([128, 16], F32, tag="gsb")
                rz = spool.tile([128, 8], F32, tag="rz")
                nt = spool.tile([128, 4], F32, tag="nt")
                u = spool.tile([128, 4], F32, tag="u")
                hnew = spool.tile([128, 4], F32, tag="hnew")
                hnew16 = spool.tile([128, 4], F16, tag="hnew16")
                if t == 0:
                    nc.vector.tensor_add(gsb[:, 0:8], pg[:, 0:8], gb_sb[:, 0:8])
                    nc.vector.tensor_add(gsb[:, 12:16], pg[:, 12:16], gb_sb[:, 12:16])
                    nc.scalar.activation(rz[:], gsb[:, 0:8], AF.Sigmoid)
                    nc.scalar.activation(nt[:], gsb[:, 12:16], AF.Tanh)
                    # hnew = n - z*n
                    nc.vector.tensor_mul(u[:], rz[:, 4:8], nt[:])
                    nc.vector.tensor_sub(hnew[:], nt[:], u[:])
                else:
                    nc.vector.tensor_add(gsb[:], pg[:, 0:16], gb_sb[:])
                    nc.scalar.activation(rz[:], gsb[:, 0:8], AF.Sigmoid)
                    nc.vector.tensor_mul(u[:], rz[:, 0:4], gsb[:, 8:12])
                    nc.vector.tensor_add(u[:], u[:], gsb[:, 12:16])
                    nc.scalar.activation(nt[:], u[:], AF.Tanh)
                    # hnew = n + z*(hidden_own - n)
                    nc.vector.tensor_sub(u[:], hp_prev[:], nt[:])
                    nc.vector.tensor_mul(u[:], rz[:, 4:8], u[:])
                    nc.vector.tensor_add(hnew[:], nt[:], u[:])
                nc.vector.tensor_copy(hnew16[:], hnew[:])

                # --- block-diag transforms ---
                pbd = pbdp.tile([128, 4], F32, tag="pbd")
                if t < t_steps - 1:
                    ptp = ptpp.tile([4, 4], F32, tag="ptp")
                for m in range(4):
                    nc.tensor.matmul(pbd[:, m:m + 1], bd_sb[:],
                                     hnew16[:, m:m + 1],
                                     start=(m == 0), stop=(m == 3))
                if t < t_steps - 1:
                    for m in range(4):
                        nc.tensor.matmul(ptp[:, m:m + 1], bd2_sb[:],
                                         hnew16[:, m:m + 1],
                                         start=(m == 0), stop=(m == 3))
                hp = hpool.tile([128, 4], F32, tag="hp")
                if t < t_steps - 1:
                    tp = spool.tile([4, 4], F32, tag="tp")
                nc.vector.tensor_scalar_add(hp[:], pbd[:, 0:4], b2t_sb[:, 0:1])
                if t < t_steps - 1:
                    nc.vector.tensor_scalar_add(tp[:], ptp[:, 0:4], cons_sb[0:4, 0:1])

                # --- all-gather (hidden'[512], temp'[16]) ---
                bi, bo = ag_in[t % 2], ag_out[t % 2]
                if debug_taps and t == 0:
                    nc.sync.dma_start(dbg_hp[:, :], hp[:])
                    nc.sync.dma_start(dbg_g[:, :], gsb[:])
                hp_prev = hp
                if t < t_steps - 1:
                    nc.sync.dma_start(bi[0:512].rearrange("(p m) -> p m", m=4), hp[:])
                    nc.sync.dma_start(bi[512:528].rearrange("(q m) -> q m", m=4), tp[:])
                    if skip_ag:
                        nc.sync.dma_start(bo[0, :], bi[:])
                        nc.sync.dma_start(bo[4, :], bi[:])
                    else:
                        nc.gpsimd.collective_compute(
                            "AllGather", mybir.AluOpType.bypass, replica_groups=RG,
                            ins=[bi.opt()], outs=[bo.opt()])
                    hidf = spool.tile([128, 8, 4], F32, tag="hidf")
                    nc.sync.dma_start(
                        hidf[:],
                        bo[:, 0:512].rearrange("c (p m) -> p c m", m=4))
                    hid16 = hpool.tile([128, 32], F16, tag="hid16")
                    nc.vector.tensor_copy(hid16[:], hidf[:])
                    if t >= T_IN - 2:
                        td = tmp_dram[t % 2]
                        nc.sync.dma_start(td.rearrange("(c s) -> c s", c=8),
                                          bo[:, 512:528])
                        tmpf = spool.tile([128, 1], F32, tag="tmpf")
                        nc.sync.dma_start(tmpf[:], td)
                        tmp16 = hpool.tile([128, 1], F16, tag="tmp16")
                        nc.vector.tensor_copy(tmp16[:], tmpf[:])

            # ---- head ----
            # stage 1 (row-sharded): partial h0a[1024] from own hidden rows,
            # AllReduce in p-major layout (idx = 8p + m), bias added after.
            p1 = pgp.tile([128, 8], F32, tag="pg")
            for k in range(4):
                for m in range(8):
                    nc.tensor.matmul(
                        p1[:, m:m + 1],
                        wh1_sb[:, (k * 1024 + m * 128):(k * 1024 + m * 128 + 128)],
                        hp_prev[:, k:k + 1],
                        start=(k == 0 and m == 0), stop=(k == 3 and m == 7))
            v1 = spool.tile([128, 8], F32, tag="hv")
            nc.vector.tensor_copy(v1[:], p1[:, 0:8])
            if debug_taps:
                nc.sync.dma_start(dbg_v1[:, :], v1[:])
            nc.sync.dma_start(ar_in[:].rearrange("(p m) -> p m", m=8), v1[:])
            nc.gpsimd.collective_compute("AllReduce", mybir.AluOpType.add,
                                         replica_groups=RG,
                                         ins=[ar_in.opt()], outs=[ar_out.opt()])
            hv1 = spool.tile([128, 8], F32, tag="hg")
            nc.sync.dma_start(hv1[:], ar_out[:].rearrange("(p m) -> p m", m=8))
            nc.vector.tensor_add(hv1[:], hv1[:], bh1f_sb[:])
            if debug_taps:
                nc.sync.dma_start(dbg_hv1[:, :], hv1[:])

            # stage 2 (col-sharded): h0b = h0a @ Wh2a_top + (table-fold bias)
            p2 = pgp.tile([128, 8], F32, tag="pg")
            for k in range(8):
                nc.tensor.matmul(p2[:, 0:1], wh2a_sb[:, k * 128:(k + 1) * 128],
                                 hv1[:, k:k + 1], start=(k == 0), stop=(k == 7))
            v2 = spool.tile([128, 1], F32, tag="hv2")
            nc.vector.tensor_scalar_add(v2[:], p2[:, 0:1], bs2_sb[:, 0:1])
            nc.sync.dma_start(agh_in[0][:], v2[:])
            nc.gpsimd.collective_compute("AllGather", mybir.AluOpType.bypass,
                                         replica_groups=RG,
                                         ins=[agh_in[0].opt()], outs=[agh_out[0].opt()])
            hv2 = spool.tile([128, 8], F32, tag="hg")
            for c in range(8):
                nc.sync.dma_start(hv2[:, c:c + 1], agh_out[0][c, :])

            # stage 3: h0c = h0b @ Wh3 + bh3
            p3 = pgp.tile([128, 8], F32, tag="pg")
            for k in range(8):
                nc.tensor.matmul(p3[:, 0:1], wh3_sb[:, k * 128:(k + 1) * 128],
                                 hv2[:, k:k + 1], start=(k == 0), stop=(k == 7))
            v3 = spool.tile([128, 1], F32, tag="hv2")
            nc.vector.tensor_scalar_add(v3[:], p3[:, 0:1], bs3_sb[:, 0:1])
            nc.sync.dma_start(agh_in[1][:], v3[:])
            nc.gpsimd.collective_compute("AllGather", mybir.AluOpType.bypass,
                                         replica_groups=RG,
                                         ins=[agh_in[1].opt()], outs=[agh_out[1].opt()])
            hv3 = spool.tile([128, 8], F32, tag="hg")
            for c in range(8):
                nc.sync.dma_start(hv3[:, c:c + 1], agh_out[1][c, :])

            # stage 4: out slice = h0c @ Whh2[:, own 256 cols] + bhh2 slice
            p4 = pbdp.tile([128, 2], F32, tag="pbd")
            for k in range(8):
                for m in range(2):
                    nc.tensor.matmul(
                        p4[:, m:m + 1],
                        whh2_sb[:, (k * 256 + m * 128):(k * 256 + m * 128 + 128)],
                        hv3[:, k:k + 1],
                        start=(k == 0 and m == 0), stop=(k == 7 and m == 1))
            v4 = spool.tile([128, 2], F32, tag="v4")
            nc.vector.tensor_add(v4[:], p4[:, 0:2], bs4_sb[:])
            # zout stored p-major: zout[2p + m] = v4[p, m]
            nc.sync.dma_start(zout[:].rearrange("(p m) -> p m", m=2), v4[:])

    nc.compile()
    return nc


def prep_inputs(inputs):
    """Host-side preprocessing: exact weight folding + per-core sharding."""
    f32 = np.float32
    h = np.asarray(inputs["h"], f32)
    W1 = np.asarray(inputs["W1"], f32)
    b1 = np.asarray(inputs["b1"], f32)
    W2 = np.asarray(inputs["W2"], f32)
    b2 = np.asarray(inputs["b2"], f32)
    Wt = np.asarray(inputs["Wt"], f32)
    bt = np.asarray(inputs["bt"], f32)
    Wh2i = np.asarray(inputs["Wh2i"], f32)
    bh2i = np.asarray(inputs["bh2i"], f32)
    Wih = np.asarray(inputs["Wih"], f32)
    Whh = np.asarray(inputs["Whh"], f32)
    bih = np.asarray(inputs["bih"], f32)
    bhh = np.asarray(inputs["bhh"], f32)
    Wh1 = np.asarray(inputs["Wh1"], f32)
    bh1 = np.asarray(inputs["bh1"], f32)
    Wh2a = np.asarray(inputs["Wh2a"], f32)
    bh2a = np.asarray(inputs["bh2a"], f32)
    Wh3 = np.asarray(inputs["Wh3"], f32)
    bh3 = np.asarray(inputs["bh3"], f32)
    Whh2 = np.asarray(inputs["Whh2"], f32)
    bhh2 = np.asarray(inputs["bhh2"], f32)

    Wih3 = Wih.reshape(NODES, LATENT, 3 * GRU)
    A = np.einsum("l,nlj->nj", W1[0], Wih3)
    cI = np.einsum("l,nlj->j", b1, Wih3) + bih

    # node permutation induced by the p-major temp bounce layout:
    # tmpf[p] = temp'[sigma(p)], sigma(16c + 4q + m) = 16c + 4m + q
    nn = np.arange(NODES)
    sigma = (nn // 16) * 16 + (nn % 4) * 4 + (nn % 16) // 4
    A = A[sigma, :]

    table = h[0, 14:21, 0].reshape(1, 7) @ Wt + bt
    cT = (table @ Wh2a[HID:] + bh2a)[0]

    bd = np.kron(np.eye(4, dtype=f32), W2).astype(np.float16)
    bd2 = np.kron(np.eye(4, dtype=f32), Wh2i).astype(np.float16)
    b2t = np.tile(b2, 4)[:, None].astype(f32)
    h16 = np.ascontiguousarray(h[:, :, 0].T[sigma, :]).astype(np.float16)
    consts = np.full((128, 1), float(bh2i[0]), f32)

    def lhsT_layout(w):  # [K, M] -> [128, (K/128)*M] k-tile-major
        K, M = w.shape
        return np.ascontiguousarray(
            w.reshape(K // 128, 128, M).transpose(1, 0, 2).reshape(128, -1))

    in_maps = []
    for c in range(NC):
        cols = np.concatenate([np.arange(g * GRU + c * CPG, g * GRU + (c + 1) * CPG)
                               for g in range(3)])
        whh_c = lhsT_layout(Whh[:, cols]).astype(np.float16)
        a_c = np.ascontiguousarray(A[:, cols]).astype(np.float16)

        comb = (cI + bhh)[cols]
        gb = np.zeros((128, 16), f32)
        gb[:, 0:8] = comb[0:1024].reshape(8, 128).T
        gb[:, 8:12] = bhh[cols][1024:1536].reshape(4, 128).T
        gb[:, 12:16] = cI[cols][1024:1536].reshape(4, 128).T

        sl128 = slice(128 * c, 128 * (c + 1))
        sl256 = slice(256 * c, 256 * (c + 1))
        in_maps.append({
            "whh": whh_c, "a16": a_c, "gb": gb, "b2t": b2t,
            "bd": bd, "bd2": bd2, "h16": h16, "consts": consts,
            "wh1": lhsT_layout(Wh1[512 * c:512 * (c + 1), :]).astype(f32),
            "bh1f": np.ascontiguousarray(bh1.reshape(8, 128).T).astype(f32),
            "wh2a": lhsT_layout(Wh2a[:HID, sl128]).astype(f32),
            "bs2": cT[sl128][:, None].astype(f32),
            "wh3": lhsT_layout(Wh3[:, sl128]).astype(f32),
            "bs3": bh3[sl128][:, None].astype(f32),
            "whh2": lhsT_layout(Whh2[:, sl256]).astype(f32),
            "bs4": np.ascontiguousarray(bhh2[sl256].reshape(2, 128).T).astype(f32),
        })
    return in_maps


_NC_CACHE = {}


def get_nc(t_steps=T):
    if t_steps not in _NC_CACHE:
        _NC_CACHE[t_steps] = build(t_steps)
    return _NC_CACHE[t_steps]


def kernel(**inputs):
    nc = get_nc(T)
    in_maps = prep_inputs(inputs)
    res = bass_utils.run_bass_kernel_spmd(nc, in_maps, core_ids=list(range(NC)))
    # zout_c[2p + m] = out[256c + 128m + p]
    z = np.concatenate(
        [res.results[c]["zout"].reshape(128, 2).T.reshape(-1) for c in range(NC)])
    return z[:HID].reshape(1, HID), z[HID:].reshape(1, HID)


# revision 22
# speedup vs baseline: 1.0064x; 1.0064x over previous
"""Trainium2 Bass kernel for nn_GAT_49168785605294.

GRU(4096) recurrence, T=64 steps, batch=1, followed by a 4-layer MLP head.

Restructuring (exact math):
  - x = (temp @ W1 + b1).flatten() is rank-1 in temp  ->  fold W1/b1 into Wih:
        gi = temp @ A + cI,   A[n,j] = sum_l W1[0,l] Wih[32n+l, j]   (A: [128, 12288])
    This removes the 201MB Wih entirely (32x less matvec work for the i-gates).
  - Gate-dim (3*4096) column-sharded across 8 cores (512 cols per gate per core).
    Whh shard (4096 x 1536, fp16) is SBUF-resident on each core.
  - Per step: PE matvec in "transposed" orientation (out tiles [128,1], weights
    as stationary lhsT) so gates land partition-major for DVE/ACT; block-diag
    W2 / Wh2i transforms as tiny matmuls; AllGather of (hidden'[512], temp'[16])
    per step across the 8 cores.
  - Head (Wh1/Wh2a/Wh3/Whh2) column-sharded, fp32, with 3 small AllGathers;
    the final [2048] output is assembled host-side from per-core shards.
"""
import numpy as np

import concourse.bass as bass
import concourse.mybir as mybir
import concourse.tile as tile
from concourse import bacc, bass_utils

F32 = mybir.dt.float32
F16 = mybir.dt.float16

NC = 8            # cores
NODES = 128
LATENT = 32
GRU = 4096        # = NODES * LATENT
HID = 1024
T_IN = 32
T = 64
CPG = 512         # columns per gate per core
MT = 12           # m-tiles per core per step (1536 / 128)
KT = 32           # k-tiles of the 4096 hidden dim
WCOLS = KT * 3 * CPG  # 49152 fp16 free elems of whh per partition

AF = mybir.ActivationFunctionType


def build(t_steps=T, debug_taps=False, skip_whh=False, skip_ag=False):
    nc = bacc.Bacc("TRN2", target_bir_lowering=False, debug=False,
                   enable_asserts=False, num_devices=NC)

    # ---- kernel I/O (per core) ----
    din = {}
    def inp(name, shape, dt):
        din[name] = nc.dram_tensor(name, list(shape), dt, kind="ExternalInput").ap()
    inp("whh", [128, WCOLS], F16)       # Whh shard, k-tile-major lhsT layout
    inp("a16", [128, 3 * CPG], F16)     # A shard
    inp("gb", [128, 16], F32)           # gate biases in psum_g layout
    inp("b2t", [128, 1], F32)           # tiled b2
    inp("bd", [128, 128], F16)          # I4 (x) W2
    inp("bd2", [128, 4], F16)           # I4 (x) Wh2i
    inp("h16", [128, T_IN], F16)        # teacher-forcing inputs, h16[n,t]
    inp("wh1", [128, 32 * 128], F32)    # head stage 1 row-shard (lhsT layout)
    inp("bh1f", [128, 8], F32)          # bh1 in p-major layout
    inp("wh2a", [128, 8 * 128], F32)
    inp("bs2", [128, 1], F32)           # includes folded table @ Wh2a_bot
    inp("wh3", [128, 8 * 128], F32)
    inp("bs3", [128, 1], F32)
    inp("whh2", [128, 8 * 256], F32)
    inp("bs4", [128, 2], F32)
    inp("consts", [128, 1], F32)        # bh2i broadcast
    zout = nc.dram_tensor("zout", [256], F32, kind="ExternalOutput").ap()
    if debug_taps:
        dbg_hp = nc.dram_tensor("dbg_hp", [128, 4], F32, kind="ExternalOutput").ap()
        dbg_g = nc.dram_tensor("dbg_g", [128, 16], F32, kind="ExternalOutput").ap()
        dbg_hv1 = nc.dram_tensor("dbg_hv1", [128, 8], F32, kind="ExternalOutput").ap()
        dbg_v1 = nc.dram_tensor("dbg_v1", [128, 8], F32, kind="ExternalOutput").ap()

    # ---- collective buffers ----
    ag_in = [nc.dram_tensor(f"ag_in{i}", [528], F32).ap() for i in range(2)]
    ag_out = [nc.dram_tensor(f"ag_out{i}", [8, 528], F32, addr_space="Shared").ap()
              for i in range(2)]
    tmp_dram = [nc.dram_tensor(f"tmp_dram{i}", [128], F32).ap() for i in range(2)]
    ar_in = nc.dram_tensor("ar_in", [1024], F32).ap()
    ar_out = nc.dram_tensor("ar_out", [1024], F32, addr_space="Shared").ap()
    agh_in = [nc.dram_tensor(f"agh_in{i}", [128], F32).ap() for i in range(3)]
    agh_out = [nc.dram_tensor(f"agh_out{i}", [8, 128], F32, addr_space="Shared").ap()
               for i in range(3)]
    RG = [list(range(NC))]

    with tile.TileContext(nc) as tc:
        with (
            tc.tile_pool(name="wpool", bufs=1) as wpool,
            tc.tile_pool(name="cpool", bufs=1) as cpool,
            tc.tile_pool(name="spool", bufs=3) as spool,
            tc.tile_pool(name="hpool", bufs=2) as hpool,
            tc.tile_pool(name="pg", bufs=2, space="PSUM") as pgp,
            tc.tile_pool(name="pbd", bufs=2, space="PSUM") as pbdp,
            tc.tile_pool(name="ptp", bufs=2, space="PSUM") as ptpp,
        ):
            # ---- load weights/consts to SBUF ----
            whh_sb = wpool.tile([128, WCOLS], F16, tag="whh")
            NSPLIT = 16
            csz = WCOLS // NSPLIT
            for i in range(NSPLIT):
                nc.sync.dma_start(whh_sb[:, i * csz:(i + 1) * csz],
                                  din["whh"][:, i * csz:(i + 1) * csz])
            a_sb = cpool.tile([128, 3 * CPG], F16, tag="a16")
            nc.sync.dma_start(a_sb[:], din["a16"][:])
            gb_sb = cpool.tile([128, 16], F32, tag="gb")
            nc.sync.dma_start(gb_sb[:], din["gb"][:])
            b2t_sb = cpool.tile([128, 1], F32, tag="b2t")
            nc.sync.dma_start(b2t_sb[:], din["b2t"][:])
            bd_sb = cpool.tile([128, 128], F16, tag="bd")
            nc.sync.dma_start(bd_sb[:], din["bd"][:])
            bd2_sb = cpool.tile([128, 4], F16, tag="bd2")
            nc.sync.dma_start(bd2_sb[:], din["bd2"][:])
            h16_sb = cpool.tile([128, T_IN], F16, tag="h16")
            nc.sync.dma_start(h16_sb[:], din["h16"][:])
            cons_sb = cpool.tile([128, 1], F32, tag="consts")
            nc.sync.dma_start(cons_sb[:], din["consts"][:])

            wh1_sb = wpool.tile([128, 32 * 128], F32, tag="wh1")
            for i in range(8):
                nc.sync.dma_start(wh1_sb[:, i * 512:(i + 1) * 512],
                                  din["wh1"][:, i * 512:(i + 1) * 512])
            wh2a_sb = wpool.tile([128, 8 * 128], F32, tag="wh2a")
            nc.sync.dma_start(wh2a_sb[:], din["wh2a"][:])
            wh3_sb = wpool.tile([128, 8 * 128], F32, tag="wh3")
            nc.sync.dma_start(wh3_sb[:], din["wh3"][:])
            whh2_sb = wpool.tile([128, 8 * 256], F32, tag="whh2")
            for i in range(4):
                nc.sync.dma_start(whh2_sb[:, i * 512:(i + 1) * 512],
                                  din["whh2"][:, i * 512:(i + 1) * 512])
            bh1f_sb = cpool.tile([128, 8], F32, tag="bh1f")
            nc.sync.dma_start(bh1f_sb[:], din["bh1f"][:])
            bs2_sb = cpool.tile([128, 1], F32, tag="bs2")
            nc.sync.dma_start(bs2_sb[:], din["bs2"][:])
            bs3_sb = cpool.tile([128, 1], F32, tag="bs3")
            nc.sync.dma_start(bs3_sb[:], din["bs3"][:])
            bs4_sb = cpool.tile([128, 2], F32, tag="bs4")
            nc.sync.dma_start(bs4_sb[:], din["bs4"][:])

            # ---- recurrence ----
            hp_prev = None        # [128,4] f32, own shard of current hidden
            hid16 = None          # [128,32] f16, full hidden (rhs for Whh MMs)
            tmp16 = None          # [128,1]  f16, full temp

            for t in range(t_steps):
                pg = pgp.tile([128, 16], F32, tag="pg")
                # --- matvec into psum_g ---
                # cols 0-3: G_r = gi_r + gh_r ; cols 4-7: G_z ; 8-11: gh_n ; 12-15: gi_n
                rhs_a = h16_sb[:, t:t + 1] if t < T_IN else tmp16[:, 0:1]
                for m in range(MT):
                    pcol = m if m < 8 else m + 4
                    nc.tensor.matmul(pg[:, pcol:pcol + 1],
                                     a_sb[:, m * 128:(m + 1) * 128], rhs_a,
                                     start=(m == 0),
                                     stop=((t == 0 or skip_whh) and m == MT - 1))
                if t > 0 and not skip_whh:
                    for m in range(MT):
                        for k in range(KT):
                            nc.tensor.matmul(
                                pg[:, m:m + 1],
                                whh_sb[:, (k * 3 * CPG + m * 128):
                                          (k * 3 * CPG + m * 128 + 128)],
                                hid16[:, k:k + 1],
                                start=False,
                                stop=(m == MT - 1 and k == KT - 1))

                # --- gates (DVE/ACT) ---
                gsb = spool.tile([128, 16], F32, tag="gsb")
                rz = spool.tile([128, 8], F32, tag="rz")
                nt = spool.tile([128, 4], F32, tag="nt")
                u = spool.tile([128, 4], F32, tag="u")
                hnew = spool.tile([128, 4], F32, tag="hnew")
                hnew16 = spool.tile([128, 4], F16, tag="hnew16")
                if t == 0:
                    nc.vector.tensor_add(gsb[:, 0:8], pg[:, 0:8], gb_sb[:, 0:8])
                    nc.vector.tensor_add(gsb[:, 12:16], pg[:, 12:16], gb_sb[:, 12:16])
                    nc.scalar.activation(rz[:], gsb[:, 0:8], AF.Sigmoid)
                    nc.scalar.activation(nt[:], gsb[:, 12:16], AF.Tanh)
                    # hnew = n - z*n
                    nc.vector.tensor_mul(u[:], rz[:, 4:8], nt[:])
                    nc.vector.tensor_sub(hnew[:], nt[:], u[:])
                else:
                    nc.vector.tensor_add(gsb[:], pg[:, 0:16], gb_sb[:])
                    nc.scalar.activation(rz[:], gsb[:, 0:8], AF.Sigmoid)
                    nc.vector.tensor_mul(u[:], rz[:, 0:4], gsb[:, 8:12])
                    nc.vector.tensor_add(u[:], u[:], gsb[:, 12:16])
                    nc.scalar.activation(nt[:], u[:], AF.Tanh)
                    # hnew = n + z*(hidden_own - n)
                    nc.vector.tensor_sub(u[:], hp_prev[:], nt[:])
                    nc.vector.tensor_mul(u[:], rz[:, 4:8], u[:])
                    nc.vector.tensor_add(hnew[:], nt[:], u[:])
                nc.vector.tensor_copy(hnew16[:], hnew[:])

                # --- block-diag transforms ---
                pbd = pbdp.tile([128, 4], F32, tag="pbd")
                if t < t_steps - 1:
                    ptp = ptpp.tile([4, 4], F32, tag="ptp")
                for m in range(4):
                    nc.tensor.matmul(pbd[:, m:m + 1], bd_sb[:],
                                     hnew16[:, m:m + 1],
                                     start=(m == 0), stop=(m == 3))
                if t < t_steps - 1:
                    for m in range(4):
                        nc.tensor.matmul(ptp[:, m:m + 1], bd2_sb[:],
                                         hnew16[:, m:m + 1],
                                         start=(m == 0), stop=(m == 3))
                hp = hpool.tile([128, 4], F32, tag="hp")
                if t < t_steps - 1:
                    tp = spool.tile([4, 4], F32, tag="tp")
                nc.vector.tensor_scalar_add(hp[:], pbd[:, 0:4], b2t_sb[:, 0:1])
                if t < t_steps - 1:
                    nc.vector.tensor_scalar_add(tp[:], ptp[:, 0:4], cons_sb[0:4, 0:1])

                # --- all-gather (hidden'[512], temp'[16]) ---
                bi, bo = ag_in[t % 2], ag_out[t % 2]
                if debug_taps and t == 0:
                    nc.sync.dma_start(dbg_hp[:, :], hp[:])
                    nc.sync.dma_start(dbg_g[:, :], gsb[:])
                hp_prev = hp
                if t < t_steps - 1:
                    nc.sync.dma_start(bi[0:512].rearrange("(p m) -> p m", m=4), hp[:])
                    nc.sync.dma_start(bi[512:528].rearrange("(q m) -> q m", m=4), tp[:])
                    if skip_ag:
                        nc.sync.dma_start(bo[0, :], bi[:])
                        nc.sync.dma_start(bo[4, :], bi[:])
                    else:
                        nc.gpsimd.collective_compute(
                            "AllGather", mybir.AluOpType.bypass, replica_groups=RG,
                            ins=[bi.opt()], outs=[bo.opt()])
                    hidf = spool.tile([128, 8, 4], F32, tag="hidf")
                    nc.sync.dma_start(
                        hidf[:],
                        bo[:, 0:512].rearrange("c (p m) -> p c m", m=4))
                    hid16 = hpool.tile([128, 32], F16, tag="hid16")
                    nc.vector.tensor_copy(hid16[:], hidf[:])
                    if t >= T_IN - 2:
                        td = tmp_dram[t % 2]
                        nc.sync.dma_start(td.rearrange("(c s) -> c s", c=8),
                                          bo[:, 512:528])
                        tmpf = spool.tile([128, 1], F32, tag="tmpf")
                        nc.sync.dma_start(tmpf[:], td)
                        tmp16 = hpool.tile([128, 1], F16, tag="tmp16")
                        nc.vector.tensor_copy(tmp16[:], tmpf[:])

            # ---- head ----
            # stage 1 (row-sharded): partial h0a[1024] from own hidden rows,
            # AllReduce in p-major layout (idx = 8p + m), bias added after.
            p1 = pgp.tile([128, 8], F32, tag="pg")
            for k in range(4):
                for m in range(8):
                    nc.tensor.matmul(
                        p1[:, m:m + 1],
                        wh1_sb[:, (k * 1024 + m * 128):(k * 1024 + m * 128 + 128)],
                        hp_prev[:, k:k + 1],
                        start=(k == 0 and m == 0), stop=(k == 3 and m == 7))
            v1 = spool.tile([128, 8], F32, tag="hv")
            nc.vector.tensor_copy(v1[:], p1[:, 0:8])
            if debug_taps:
                nc.sync.dma_start(dbg_v1[:, :], v1[:])
            nc.sync.dma_start(ar_in[:].rearrange("(p m) -> p m", m=8), v1[:])
            nc.gpsimd.collective_compute("AllReduce", mybir.AluOpType.add,
                                         replica_groups=RG,
                                         ins=[ar_in.opt()], outs=[ar_out.opt()])
            hv1 = spool.tile([128, 8], F32, tag="hg")
            nc.sync.dma_start(hv1[:], ar_out[:].rearrange("(p m) -> p m", m=8))
            nc.vector.tensor_add(hv1[:], hv1[:], bh1f_sb[:])
            if debug_taps:
                nc.sync.dma_start(dbg_hv1[:, :], hv1[:])

            # stage 2 (col-sharded): h0b = h0a @ Wh2a_top + (table-fold bias)
            p2 = pgp.tile([128, 8], F32, tag="pg")
            for k in range(8):
                nc.tensor.matmul(p2[:, 0:1], wh2a_sb[:, k * 128:(k + 1) * 128],
                                 hv1[:, k:k + 1], start=(k == 0), stop=(k == 7))
            v2 = spool.tile([128, 1], F32, tag="hv2")
            nc.vector.tensor_scalar_add(v2[:], p2[:, 0:1], bs2_sb[:, 0:1])
            nc.sync.dma_start(agh_in[0][:], v2[:])
            nc.gpsimd.collective_compute("AllGather", mybir.AluOpType.bypass,
                                         replica_groups=RG,
                                         ins=[agh_in[0].opt()], outs=[agh_out[0].opt()])
            hv2 = spool.tile([128, 8], F32, tag="hg")
            for c in range(8):
                nc.sync.dma_start(hv2[:, c:c + 1], agh_out[0][c, :])

            # stage 3: h0c = h0b @ Wh3 + bh3
            p3 = pgp.tile([128, 8], F32, tag="pg")
            for k in range(8):
                nc.tensor.matmul(p3[:, 0:1], wh3_sb[:, k * 128:(k + 1) * 128],
                                 hv2[:, k:k + 1], start=(k == 0), stop=(k == 7))
            v3 = spool.tile([128, 1], F32, tag="hv2")
            nc.vector.tensor_scalar_add(v3[:], p3[:, 0:1], bs3_sb[:, 0:1])
            nc.sync.dma_start(agh_in[1][:], v3[:])
            nc.gpsimd.collective_compute("AllGather", mybir.AluOpType.bypass,
                                         replica_groups=RG,
                                         ins=[agh_in[1].opt()], outs=[agh_out[1].opt()])
            hv3 = spool.tile([128, 8], F32, tag="hg")
            for c in range(8):
                nc.sync.dma_start(hv3[:, c:c + 1], agh_out[1][c, :])

            # stage 4: out slice = h0c @ Whh2[:, own 256 cols] + bhh2 slice
            p4 = pbdp.tile([128, 2], F32, tag="pbd")
            for k in range(8):
                for m in range(2):
                    nc.tensor.matmul(
                        p4[:, m:m + 1],
                        whh2_sb[:, (k * 256 + m * 128):(k * 256 + m * 128 + 128)],
                        hv3[:, k:k + 1],
                        start=(k == 0 and m == 0), stop=(k == 7 and m == 1))
            v4 = spool.tile([128, 2], F32, tag="v4")
            nc.vector.tensor_add(v4[:], p4[:, 0:2], bs4_sb[:])
            # zout stored p-major: zout[2p + m] = v4[p, m]
            nc.sync.dma_start(zout[:].rearrange("(p m) -> p m", m=2), v4[:])

    nc.compile()
    return nc


def prep_inputs(inputs):
    """Host-side preprocessing: exact weight folding + per-core sharding."""
    f32 = np.float32
    h = np.asarray(inputs["h"], f32)
    W1 = np.asarray(inputs["W1"], f32)
    b1 = np.asarray(inputs["b1"], f32)
    W2 = np.asarray(inputs["W2"], f32)
    b2 = np.asarray(inputs["b2"], f32)
    Wt = np.asarray(inputs["Wt"], f32)
    bt = np.asarray(inputs["bt"], f32)
    Wh2i = np.asarray(inputs["Wh2i"], f32)
    bh2i = np.asarray(inputs["bh2i"], f32)
    Wih = np.asarray(inputs["Wih"], f32)
    Whh = np.asarray(inputs["Whh"], f32)
    bih = np.asarray(inputs["bih"], f32)
    bhh = np.asarray(inputs["bhh"], f32)
    Wh1 = np.asarray(inputs["Wh1"], f32)
    bh1 = np.asarray(inputs["bh1"], f32)
    Wh2a = np.asarray(inputs["Wh2a"], f32)
    bh2a = np.asarray(inputs["bh2a"], f32)
    Wh3 = np.asarray(inputs["Wh3"], f32)
    bh3 = np.asarray(inputs["bh3"], f32)
    Whh2 = np.asarray(inputs["Whh2"], f32)
    bhh2 = np.asarray(inputs["bhh2"], f32)

    Wih3 = Wih.reshape(NODES, LATENT, 3 * GRU)
    A = np.einsum("l,nlj->nj", W1[0], Wih3)
    cI = np.einsum("l,nlj->j", b1, Wih3) + bih

    # node permutation induced by the p-major temp bounce layout:
    # tmpf[p] = temp'[sigma(p)], sigma(16c + 4q + m) = 16c + 4m + q
    nn = np.arange(NODES)
    sigma = (nn // 16) * 16 + (nn % 4) * 4 + (nn % 16) // 4
    A = A[sigma, :]

    table = h[0, 14:21, 0].reshape(1, 7) @ Wt + bt
    cT = (table @ Wh2a[HID:] + bh2a)[0]

    bd = np.kron(np.eye(4, dtype=f32), W2).astype(np.float16)
    bd2 = np.kron(np.eye(4, dtype=f32), Wh2i).astype(np.float16)
    b2t = np.tile(b2, 4)[:, None].astype(f32)
    h16 = np.ascontiguousarray(h[:, :, 0].T[sigma, :]).astype(np.float16)
    consts = np.full((128, 1), float(bh2i[0]), f32)

    def lhsT_layout(w):  # [K, M] -> [128, (K/128)*M] k-tile-major
        K, M = w.shape
        return np.ascontiguousarray(
            w.reshape(K // 128, 128, M).transpose(1, 0, 2).reshape(128, -1))

    in_maps = []
    for c in range(NC):
        cols = np.concatenate([np.arange(g * GRU + c * CPG, g * GRU + (c + 1) * CPG)
                               for g in range(3)])
        whh_c = lhsT_layout(Whh[:, cols]).astype(np.float16)
        a_c = np.ascontiguousarray(A[:, cols]).astype(np.float16)

        comb = (cI + bhh)[cols]
        gb = np.zeros((128, 16), f32)
        gb[:, 0:8] = comb[0:1024].reshape(8, 128).T
        gb[:, 8:12] = bhh[cols][1024:1536].reshape(4, 128).T
        gb[:, 12:16] = cI[cols][1024:1536].reshape(4, 128).T

        sl128 = slice(128 * c, 128 * (c + 1))
        sl256 = slice(256 * c, 256 * (c + 1))
        in_maps.append({
            "whh": whh_c, "a16": a_c, "gb": gb, "b2t": b2t,
            "bd": bd, "bd2": bd2, "h16": h16, "consts": consts,
            "wh1": lhsT_layout(Wh1[512 * c:512 * (c + 1), :]).astype(f32),
            "bh1f": np.ascontiguousarray(bh1.reshape(8, 128).T).astype(f32),
            "wh2a": lhsT_layout(Wh2a[:HID, sl128]).astype(f32),
            "bs2": cT[sl128][:, None].astype(f32),
            "wh3": lhsT_layout(Wh3[:, sl128]).astype(f32),
            "bs3": bh3[sl128][:, None].astype(f32),
            "whh2": lhsT_layout(Whh2[:, sl256]).astype(f32),
            "bs4": np.ascontiguousarray(bhh2[sl256].reshape(2, 128).T).astype(f32),
        })
    return in_maps


_NC_CACHE = {}


def get_nc(t_steps=T):
    if t_steps not in _NC_CACHE:
        _NC_CACHE[t_steps] = build(t_steps)
    return _NC_CACHE[t_steps]


def kernel(**inputs):
    nc = get_nc(T)
    in_maps = prep_inputs(inputs)
    res = bass_utils.run_bass_kernel_spmd(nc, in_maps, core_ids=list(range(NC)))
    # zout_c[2p + m] = out[256c + 128m + p]
    z = np.concatenate(
        [res.results[c]["zout"].reshape(128, 2).T.reshape(-1) for c in range(NC)])
    return z[:HID].reshape(1, HID), z[HID:].reshape(1, HID)
